# revision 44
# baseline (speedup 1.0000x reference)
# Trainium2 Bass kernel for nn_Attention_48052094107920 (sparse_attention).
#
# Math (see reference):
#   q,k: GH=3 global heads of dim 64; v: LH=12 local heads of dim 64
#   S_g = (x Wq)_g (x Wk)_g^T * scale                  [B,3,N,N]
#   mw  = (masks @ mask_proj).reshape(N,N,3,12)
#   A_h = sum_g S_g * mw[:,:,g,h]                      [B,12,N,N]
#   out = softmax_k(A_h) @ v_h  -> output projection + bias
#
# Sharding: core c = (head-group c//2, query-half c%2). Each core processes
# all 8 batches for its 3 local heads and its 289-token query half and emits
# partial (pre-bias) projection outputs; the host sums the 4 head-group
# partials and adds proj_b. mw (the replicated mask_weights of the sharding
# hint) is computed on the host and DMA'd in per-core slices.
#
# Device-side design:
#   - "k-major" score layout: score tiles are S^T[k, q] (k on partitions) so
#     p @ v needs no transposes and the output projection receives its lhsT
#     (= o^T) directly from PSUM.
#   - Per-core token permutation puts this core's q-half at columns 0:289 of
#     x-hat^T, so q projections slice the same SBUF tile as k/v (no extra
#     DMA). mw is host-built in the same permuted (k, q) indexing.
#   - v-hat columns are interleaved [v_h | ones]; the ones column (memset,
#     not projected) produces the softmax denominator Z during p@v.
#   - softmax skips max-subtraction (logits are O(5)); 1/Z is folded in
#     after p@v; padded k-rows are killed with a -30 exp bias.
#   - engine balance: PE does all matmuls; Act does exp + score/osbz copies;
#     DVE does most of the mask-mix + reciprocal + normalize; Pool (gpsimd)
#     does the rest of the mix, phase-A copies and the Z broadcast. The
#     output projection DMAs straight from PSUM (f32) to DRAM.

import numpy as np
import ml_dtypes

import concourse.bass as bass
import concourse.bacc as bacc_mod
import concourse.mybir as mybir
import concourse.tile as tile
from concourse import bass_utils

BF = mybir.dt.float16  # fp16: 10-bit mantissa, same engine speed as bf16
F32 = mybir.dt.float32
AF = mybir.ActivationFunctionType
OP = mybir.AluOpType

B, N, C = 8, 577, 768
GH, LH, ML, HD = 3, 12, 3, 64
NH = 3            # heads per core
SCALE = HD ** -0.5
NP = 640          # padded token count (5 * 128)
NJ = 5            # k sub-chunks of 128
KO = 6            # contraction sub-chunks (768 = 6*128, no ones row)
QW = 289          # q-half width (577 = 289 + 288)
VW = HD + 1       # 65: v head columns + ones column
EXP_NEG = -30.0   # exp bias for padded k rows (j=4, partitions >= 65)


def build_nc3():
    nc = bacc_mod.Bacc("TRN2", target_bir_lowering=False, debug=False, num_devices=8)

    xta = nc.dram_tensor("xta", [B, 128, KO, NP], BF, kind="ExternalInput")
    wq = nc.dram_tensor("wq", [128, KO, GH * HD], BF, kind="ExternalInput")
    wk = nc.dram_tensor("wk", [128, KO, GH * HD], BF, kind="ExternalInput")
    wv = nc.dram_tensor("wv", [128, KO, NH * VW], BF, kind="ExternalInput")
    pw01 = nc.dram_tensor("pw01", [128, C], BF, kind="ExternalInput")
    pw2 = nc.dram_tensor("pw2", [64, C], BF, kind="ExternalInput")
    mw = nc.dram_tensor("mw", [128, GH * NH, NJ, QW], BF, kind="ExternalInput")
    eb = nc.dram_tensor("eb", [128, 2], F32, kind="ExternalInput")
    out = nc.dram_tensor("op", [B, QW, C], BF, kind="ExternalOutput")

    with tile.TileContext(nc) as tc, \
         tc.tile_pool(name="const", bufs=1) as cpool, \
         tc.tile_pool(name="xb", bufs=2) as xpool, \
         tc.tile_pool(name="work", bufs=3) as wpool, \
         tc.tile_pool(name="attn", bufs=3) as apool, \
         tc.tile_pool(name="psA", bufs=2, space="PSUM") as ppA, \
         tc.tile_pool(name="psSa", bufs=1, space="PSUM") as ppSa, \
         tc.tile_pool(name="psSb", bufs=1, space="PSUM") as ppSb, \
         tc.tile_pool(name="psO", bufs=2, space="PSUM") as ppO:

        # consts on the Act DMA queue so the batch-0 x DMA (sync queue) is
        # not stuck behind the large mw transfer
        wq_s = cpool.tile([128, KO, GH * HD], BF, tag="wq")
        nc.scalar.dma_start(wq_s[:], wq.ap())
        wk_s = cpool.tile([128, KO, GH * HD], BF, tag="wk")
        nc.scalar.dma_start(wk_s[:], wk.ap())
        wv_s = cpool.tile([128, KO, NH * VW], BF, tag="wv")
        nc.scalar.dma_start(wv_s[:], wv.ap())
        pw01_s = cpool.tile([128, C], BF, tag="pw01")
        pw2_s = cpool.tile([64, C], BF, tag="pw2")
        mw_s = cpool.tile([128, GH * NH, NJ, QW], BF, tag="mw")
        eb_s = cpool.tile([128, 2], F32, tag="eb")
        nc.scalar.dma_start(eb_s[:], eb.ap())
        # dummy exp: hoist the 1.3us LoadActFuncSet into the idle startup
        # window instead of delaying the first real exp
        warm = cpool.tile([1, 2], F32, tag="warm")
        nc.scalar.activation(warm[:], eb_s[0:1, :], AF.Exp)
        # (eb/weights precede mw on the Act DMA queue; mw is the big one)

        # phase A is split into parts so the emission schedule can spread
        # projection (PE) work evenly across iterations
        def a_q(st):
            xb = xpool.tile([128, KO, NP], BF, tag="xb")
            nc.sync.dma_start(xb[:], xta.ap()[st["b"]])
            st["xb"] = xb
            q01 = wpool.tile([128, QW], BF, tag="q01")
            q2 = wpool.tile([64, QW], BF, tag="q2")
            for msl, mp, dst in ((slice(0, 128), 128, q01), (slice(128, 192), 64, q2)):
                ps = ppA.tile([128, 512], F32, tag="bigA", name="psA")[:mp, :QW]
                for o in range(KO):
                    nc.tensor.matmul(ps, wq_s[:, o, msl], xb[:, o, 0:QW],
                                     start=(o == 0), stop=(o == KO - 1))
                nc.vector.tensor_copy(dst[:mp, :], ps)
            st["q01"], st["q2"] = q01, q2

        def _kproj(st, msl, mp, dst):
            # project only the 577 valid tokens; zero the pad columns so the
            # padded score rows multiply to 0 (not NaN) in the mask-mix
            for n0, n1 in ((0, 512), (512, N)):
                ps = ppA.tile([128, 512], F32, tag="bigA", name="psA")[:mp, : n1 - n0]
                for o in range(KO):
                    nc.tensor.matmul(ps, wk_s[:, o, msl], st["xb"][:, o, n0:n1],
                                     start=(o == 0), stop=(o == KO - 1))
                if mp == 128 and n0 != 0:
                    nc.scalar.copy(dst[:mp, n0:n1], ps)
                else:
                    nc.vector.tensor_copy(dst[:mp, n0:n1], ps)
            nc.gpsimd.memset(dst[:mp, N:NP], 0.0)

        def a_k01(st):
            k01 = wpool.tile([128, NP], BF, tag="k01")
            _kproj(st, slice(0, 128), 128, k01)
            st["k01"] = k01

        def a_k2(st):
            k2 = wpool.tile([64, NP], BF, tag="k2")
            _kproj(st, slice(128, 192), 64, k2)
            st["k2"] = k2

        def a_v(st):
            # v-hat (k rows, interleaved [v | ones] columns; ones via memset)
            xb = st["xb"]
            vtb = wpool.tile([128, NJ, NH * VW], BF, tag="vtb")
            for kc in range(NJ):
                ps = ppA.tile([128, 512], F32, tag="bigA", name="psA")[:, : NH * VW]
                for o in range(KO):
                    nc.tensor.matmul(ps, xb[:, o, kc * 128:(kc + 1) * 128], wv_s[:, o, :],
                                     start=(o == 0), stop=(o == KO - 1))
                nc.vector.tensor_copy(vtb[:, kc, :], ps)
            nc.gpsimd.memset(vtb[:, :, HD::VW], 1.0)
            st["vtb"] = vtb

        A_PARTS = (a_q, a_k01, a_k2, a_v)

        def phase_a(b):
            st = {"b": b}
            for p in A_PARTS:
                p(st)
            return st

        def qg(st, g):
            q01, q2 = st["q01"], st["q2"]
            return (q01[0:64], q01[64:128], q2[0:64])[g]

        def kg(st, g):
            k01, k2 = st["k01"], st["k2"]
            return (k01[0:64], k01[64:128], k2[0:64])[g]

        def scores(st):
            # S^T[k, q] per global head, PSUM split 256 + 33 so no matmul
            # output crosses a PSUM bank boundary
            ssb = wpool.tile([128, GH, NJ, QW], BF, tag="ssb")
            for g in range(GH):
                psa = ppSa.tile([128, NJ, 256], F32, tag="sa", name="psSa")
                psb = ppSb.tile([128, NJ, 33], F32, tag="sb", name="psSb")
                for j in range(NJ):
                    kj = kg(st, g)[:, j * 128:(j + 1) * 128]
                    nc.tensor.matmul(psa[:, j, :], kj, qg(st, g)[:, 0:256],
                                     start=True, stop=True)
                    nc.tensor.matmul(psb[:, j, :], kj, qg(st, g)[:, 256:QW],
                                     start=True, stop=True)
                nc.scalar.copy(ssb[:, g, :, 0:256], psa)
                nc.scalar.copy(ssb[:, g, :, 256:QW], psb)
            return ssb

        def mix_exp(ssb):
            # mask-mix (attn_h = sum_g ssb_g * mw_gh), split DVE / Pool
            es = []
            for hh in range(NH):
                at = apool.tile([128, NJ, QW], BF, tag="at")
                t1 = apool.tile([128, NJ, QW], BF, tag="t1")
                t2 = apool.tile([128, NJ, QW], BF, tag="t2")
                e = apool.tile([128, NJ, QW], BF, tag="e")
                nc.vector.tensor_mul(at[:], ssb[:, 0], mw_s[:, hh])
                nc.gpsimd.tensor_mul(t1[:], ssb[:, 1], mw_s[:, NH + hh])
                if hh < 2:
                    nc.gpsimd.tensor_mul(t2[:], ssb[:, 2], mw_s[:, 2 * NH + hh])
                else:
                    nc.vector.tensor_mul(t2[:], ssb[:, 2], mw_s[:, 2 * NH + hh])
                nc.gpsimd.tensor_tensor(out=at[:], in0=at[:], in1=t1[:], op=OP.add)
                nc.vector.tensor_add(at[:], at[:], t2[:])
                # exp(at - 5): the softmax-invariant shift keeps e and Z
                # well inside fp16 range on hardware (max logit ~= 11)
                nc.scalar.activation(e[:, 0:4], at[:, 0:4], AF.Exp, bias=eb_s[:, 1:2])
                nc.scalar.activation(e[:, 4:5], at[:, 4:5], AF.Exp, bias=eb_s[:, 0:1])
                es.append(e)
            return es

        def pv(es, vtb):
            # o^T_h (and Z in row 64) = vhat_h^T @ e
            osbz = wpool.tile([VW, NH, QW], BF, tag="osbz")
            for hh in range(NH):
                pov = ppO.tile([VW, QW], F32, tag="ov", name="psO")
                for j in range(NJ):
                    nc.tensor.matmul(pov, vtb[:, j, hh * VW:(hh + 1) * VW],
                                     es[hh][:, j, :],
                                     start=(j == 0), stop=(j == NJ - 1))
                nc.scalar.copy(osbz[:, hh], pov)
            return osbz

        def zout(b, osbz):
            # 1/Z broadcast over the 64 head-dim partitions, per head so the
            # chain pipelines with the per-head pv/osbz copies
            zrec = wpool.tile([1, NH, QW], BF, tag="zrec")
            zrep = wpool.tile([64, NH, QW], BF, tag="zrep")
            on01 = wpool.tile([128, QW], BF, tag="on01")
            on2 = wpool.tile([64, QW], BF, tag="on2")
            ons = (on01[0:64, :], on01[64:128, :], on2[:])
            with nc.allow_low_precision(reason="1/Z in fp16: Z in [1, 640], rel err ~5e-4"):
                nc.vector.reciprocal(zrec[:], osbz[HD:VW, :])
                for hh in range(NH):
                    nc.gpsimd.partition_broadcast(zrep[:, hh], zrec[0:1, hh], channels=64)
                    nc.gpsimd.tensor_mul(ons[hh], osbz[0:HD, hh], zrep[:, hh])

            # output projection (partial, pre-bias; host sums head groups)
            for q0, q1 in ((0, 128), (128, 256), (256, QW)):
                outsb = wpool.tile([128, C], BF, tag="outsb")
                for n0, n1 in ((0, 512), (512, C)):
                    ps = ppA.tile([128, 512], F32, tag="bigA", name="psA")[
                        : q1 - q0, : n1 - n0]
                    nc.tensor.matmul(ps, on01[:, q0:q1], pw01_s[:, n0:n1],
                                     start=True, stop=False)
                    nc.tensor.matmul(ps, on2[:, q0:q1], pw2_s[:, n0:n1],
                                     start=False, stop=True)
                    if n0 == 0:
                        nc.vector.tensor_copy(outsb[: q1 - q0, n0:n1], ps)
                    else:
                        nc.scalar.copy(outsb[: q1 - q0, n0:n1], ps)
                nc.sync.dma_start(out.ap()[b, q0:q1, :], outsb[: q1 - q0, :])

        # depth-4 software pipeline. Iteration b emits
        #   mix+exp(b) | scores(b+2) | phase-A parts | pv(b) | out-proj(b)
        # Scores run two batches ahead of the mix that consumes them, so the
        # mask-mix of b starts right at the top of iteration b, and the PE
        # runs scores/projections while it completes.
        st = {0: phase_a(0), 1: phase_a(1)}
        # big const DMAs ride the sync queue behind xb0/xb1 but ahead of
        # later xb's: g0/g1 mask maps first (needed by the first mix ops)
        nc.sync.dma_start(mw_s[:, 0:2 * NH], mw.ap()[:, 0:2 * NH])
        nc.sync.dma_start(mw_s[:, 2 * NH:], mw.ap()[:, 2 * NH:])
        ssbs = {0: scores(st[0])}
        st[2] = phase_a(2)
        nc.sync.dma_start(pw01_s[:], pw01.ap())
        nc.sync.dma_start(pw2_s[:], pw2.ap())
        ssbs[1] = scores(st[1])
        st[3] = phase_a(3)

        # spread the remaining phase-A parts (batches 4..7) over the
        # iterations, each finished by its scores deadline (iter m-2)
        SPREAD = {
            0: ((4, 0), (4, 1), (4, 2)),
            1: ((4, 3), (5, 0), (5, 1)),
            2: ((5, 2), (5, 3), (6, 0)),
            3: ((6, 1), (6, 2), (6, 3)),
            4: ((7, 0), (7, 1), (7, 2)),
            5: ((7, 3),),
            6: (),
            7: (),
        }
        for b in range(B):
            es = mix_exp(ssbs.pop(b))
            for m, pi in SPREAD[b]:
                if pi == 0:
                    st[m] = {"b": m}
                A_PARTS[pi](st[m])
            if b + 2 < B:
                ssbs[b + 2] = scores(st[b + 2])
            osbz = pv(es, st[b]["vtb"])
            del st[b]
            zout(b, osbz)

    nc.compile()
    return nc


def _perms():
    """Per-q-half token permutations: the core's q tokens first."""
    p0 = np.arange(N)
    p1 = np.concatenate([np.arange(QW, N), np.arange(QW)])
    return (p0, p1)


def prep_inputs3(x, masks, Wq, Wk, Wv, mask_proj, proj_w, proj_b):
    f16 = np.float16

    # mask_weights (replicated across batch, per the sharding hint)
    P = (masks.reshape(N * N, ML).astype(np.float32) @ mask_proj).reshape(
        N, N, GH, LH)

    ebp = np.full((128, 2), -5.0, np.float32)
    ebp[VW:, 0] += EXP_NEG  # j=4 chunk: k = 512 + p valid through p = 64

    perms = _perms()
    xtas = []
    mwk = []  # [s] -> [q', k'(pad 640), GH, LH] permuted mask weights
    for s in range(2):
        perm = perms[s]
        xhatT = np.zeros((B, C, NP), np.float32)
        xhatT[:, :, :N] = x[:, perm, :].transpose(0, 2, 1)
        xtas.append(np.ascontiguousarray(
            xhatT.reshape(B, KO, 128, NP).transpose(0, 2, 1, 3)).astype(f16))
        Pp = P[np.ix_(perm, perm)]                      # [q', k', GH, LH]
        Ppad = np.zeros((QW, NP, GH, LH), np.float32)
        Ppad[:, :N] = Pp[:QW]
        mwk.append(Ppad)

    def wpad(w, scale=1.0):
        return np.ascontiguousarray(
            (w * scale).reshape(KO, 128, -1).transpose(1, 0, 2)).astype(f16)

    wqp = wpad(Wq, SCALE)
    wkp = wpad(Wk)

    in_maps = []
    for c in range(8):
        hg, s = c // 2, c % 2
        H0 = NH * hg

        wvh = np.zeros((C, NH * VW), np.float32)
        for hh in range(NH):
            h = H0 + hh
            wvh[:, hh * VW:hh * VW + HD] = Wv[:, h * HD:(h + 1) * HD]
        wvp = np.ascontiguousarray(
            wvh.reshape(KO, 128, -1).transpose(1, 0, 2)).astype(f16)

        pw01p = np.ascontiguousarray(
            proj_w[H0 * HD:(H0 + 2) * HD, :]).astype(f16)
        pw2p = np.ascontiguousarray(
            proj_w[(H0 + 2) * HD:(H0 + 3) * HD, :]).astype(f16)

        # mw[p, g*NH+hh, j, q'] = P[perm[q'], perm[j*128+p], g, H0+hh]
        mwc = np.ascontiguousarray(
            mwk[s][:, :, :, H0:H0 + NH]                 # [q', k', g, hh]
            .reshape(QW, NJ, 128, GH, NH)
            .transpose(2, 3, 4, 1, 0)                   # [p, g, hh, j, q']
            .reshape(128, GH * NH, NJ, QW)).astype(f16)

        in_maps.append({
            "xta": xtas[s],
            "wq": wqp, "wk": wkp, "wv": wvp,
            "pw01": pw01p, "pw2": pw2p,
            "mw": mwc, "eb": ebp,
        })
    return in_maps


_NC3 = None


def get_nc3():
    global _NC3
    if _NC3 is None:
        _NC3 = build_nc3()
    return _NC3


def kernel_v3(x, masks, Wq, Wk, Wv, mask_proj, proj_w, proj_b):
    x = np.asarray(x, np.float32)
    in_maps = prep_inputs3(
        x, np.asarray(masks, np.float32), np.asarray(Wq, np.float32),
        np.asarray(Wk, np.float32), np.asarray(Wv, np.float32),
        np.asarray(mask_proj, np.float32), np.asarray(proj_w, np.float32),
        np.asarray(proj_b, np.float32))
    res = bass_utils.run_bass_kernel_spmd(get_nc3(), in_maps, core_ids=list(range(8)))
    # sum the 4 head-group partials per q-half, concat halves, add bias
    out = np.zeros((B, N, C), np.float32)
    for c in range(8):
        hg, s = c // 2, c % 2
        r = np.asarray(res.results[c]["op"], np.float32)
        if s == 0:
            out[:, 0:QW, :] += r
        else:
            out[:, QW:N, :] += r[:, 0:N - QW, :]
    out += np.asarray(proj_b, np.float32)
    return out.astype(np.float32)


def kernel(x, masks, Wq, Wk, Wv, mask_proj, proj_w, proj_b):
    return kernel_v3(x, masks, Wq, Wk, Wv, mask_proj, proj_w, proj_b)


if __name__ == "__main__":
    rng = np.random.default_rng(0)
    ins = {
        "x": rng.standard_normal((B, N, C)).astype(np.float32),
        "masks": rng.random((N, N, ML)).astype(np.float32),
        "Wq": (rng.standard_normal((C, GH * HD)) * 0.02).astype(np.float32),
        "Wk": (rng.standard_normal((C, GH * HD)) * 0.02).astype(np.float32),
        "Wv": (rng.standard_normal((C, C)) * 0.02).astype(np.float32),
        "mask_proj": (rng.standard_normal((ML, GH * LH)) * 0.5 + 1.0).astype(np.float32),
        "proj_w": (rng.standard_normal((C, C)) * 0.02).astype(np.float32),
        "proj_b": (rng.standard_normal(C) * 0.02).astype(np.float32),
    }
    out = kernel(**ins)
    print(out.shape, out.dtype)


# revision 49
# speedup vs baseline: 1.0286x; 1.0286x over previous
# Trainium2 Bass kernel for nn_Attention_48052094107920 (sparse_attention).
#
# Math (see reference):
#   q,k: GH=3 global heads of dim 64; v: LH=12 local heads of dim 64
#   S_g = (x Wq)_g (x Wk)_g^T * scale                  [B,3,N,N]
#   mw  = (masks @ mask_proj).reshape(N,N,3,12)
#   A_h = sum_g S_g * mw[:,:,g,h]                      [B,12,N,N]
#   out = softmax_k(A_h) @ v_h  -> output projection + bias
#
# Sharding: core c = (head-group c//2, query-half c%2). Each core processes
# all 8 batches for its 3 local heads and its 289-token query half and emits
# partial (pre-bias) projection outputs; the host sums the 4 head-group
# partials and adds proj_b. mw (the replicated mask_weights of the sharding
# hint) is computed on the host and DMA'd in per-core slices.
#
# Device-side design:
#   - "k-major" score layout: score tiles are S^T[k, q] (k on partitions) so
#     p @ v needs no transposes and the output projection receives its lhsT
#     (= o^T) directly from PSUM.
#   - Per-core token permutation puts this core's q-half at columns 0:289 of
#     x-hat^T, so q projections slice the same SBUF tile as k/v (no extra
#     DMA). mw is host-built in the same permuted (k, q) indexing.
#   - v-hat columns are interleaved [v_h | ones]; the ones column (memset,
#     not projected) produces the softmax denominator Z during p@v.
#   - softmax skips max-subtraction (logits are O(5)); 1/Z is folded in
#     after p@v; padded k-rows are killed with a -30 exp bias.
#   - engine balance: PE does all matmuls; Act does exp + score/osbz copies;
#     DVE does most of the mask-mix + reciprocal + normalize; Pool (gpsimd)
#     does the rest of the mix, phase-A copies and the Z broadcast. The
#     output projection DMAs straight from PSUM (f32) to DRAM.

import numpy as np
import ml_dtypes

import concourse.bass as bass
import concourse.bacc as bacc_mod
import concourse.mybir as mybir
import concourse.tile as tile
from concourse import bass_utils

BF = mybir.dt.float16  # fp16: 10-bit mantissa, same engine speed as bf16
F32 = mybir.dt.float32
AF = mybir.ActivationFunctionType
OP = mybir.AluOpType

B, N, C = 8, 577, 768
GH, LH, ML, HD = 3, 12, 3, 64
NH = 3            # heads per core
SCALE = HD ** -0.5
NP = 640          # padded token count (5 * 128)
NJ = 5            # k sub-chunks of 128
KO = 6            # contraction sub-chunks (768 = 6*128, no ones row)
QW = 289          # q-half width (577 = 289 + 288)
VW = HD + 1       # 65: v head columns + ones column
EXP_NEG = -30.0   # exp bias for padded k rows (j=4, partitions >= 65)


def build_nc3():
    nc = bacc_mod.Bacc("TRN2", target_bir_lowering=False, debug=False, num_devices=8)

    xta = nc.dram_tensor("xta", [B, 128, KO, NP], BF, kind="ExternalInput")
    wq = nc.dram_tensor("wq", [128, KO, GH * HD], BF, kind="ExternalInput")
    wk = nc.dram_tensor("wk", [128, KO, GH * HD], BF, kind="ExternalInput")
    wv = nc.dram_tensor("wv", [128, KO, NH * VW], BF, kind="ExternalInput")
    pw01 = nc.dram_tensor("pw01", [128, C], BF, kind="ExternalInput")
    pw2 = nc.dram_tensor("pw2", [64, C], BF, kind="ExternalInput")
    mw = nc.dram_tensor("mw", [128, GH * NH, NJ, QW], BF, kind="ExternalInput")
    eb = nc.dram_tensor("eb", [128, 2], F32, kind="ExternalInput")
    out = nc.dram_tensor("op", [B, QW, C], BF, kind="ExternalOutput")

    with tile.TileContext(nc) as tc, \
         tc.tile_pool(name="const", bufs=1) as cpool, \
         tc.tile_pool(name="xb", bufs=2) as xpool, \
         tc.tile_pool(name="work", bufs=3) as wpool, \
         tc.tile_pool(name="attn", bufs=3) as apool, \
         tc.tile_pool(name="psA", bufs=2, space="PSUM") as ppA, \
         tc.tile_pool(name="psSa", bufs=1, space="PSUM") as ppSa, \
         tc.tile_pool(name="psSb", bufs=1, space="PSUM") as ppSb, \
         tc.tile_pool(name="psO", bufs=2, space="PSUM") as ppO:

        # consts on the Act DMA queue so the batch-0 x DMA (sync queue) is
        # not stuck behind the large mw transfer
        wq_s = cpool.tile([128, KO, GH * HD], BF, tag="wq")
        nc.scalar.dma_start(wq_s[:], wq.ap())
        wk_s = cpool.tile([128, KO, GH * HD], BF, tag="wk")
        nc.scalar.dma_start(wk_s[:], wk.ap())
        wv_s = cpool.tile([128, KO, NH * VW], BF, tag="wv")
        nc.scalar.dma_start(wv_s[:], wv.ap())
        pw01_s = cpool.tile([128, C], BF, tag="pw01")
        pw2_s = cpool.tile([64, C], BF, tag="pw2")
        mw_s = cpool.tile([128, GH * NH, NJ, QW], BF, tag="mw")
        eb_s = cpool.tile([128, 2], F32, tag="eb")
        nc.scalar.dma_start(eb_s[:], eb.ap())
        # dummy exp: hoist the 1.3us LoadActFuncSet into the idle startup
        # window instead of delaying the first real exp
        warm = cpool.tile([1, 2], F32, tag="warm")
        nc.scalar.activation(warm[:], eb_s[0:1, :], AF.Exp)
        # (eb/weights precede mw on the Act DMA queue; mw is the big one)

        # phase A is split into parts so the emission schedule can spread
        # projection (PE) work evenly across iterations
        def a_q(st):
            xb = xpool.tile([128, KO, NP], BF, tag="xb")
            nc.sync.dma_start(xb[:], xta.ap()[st["b"]])
            st["xb"] = xb
            q01 = wpool.tile([128, QW], BF, tag="q01")
            q2 = wpool.tile([64, QW], BF, tag="q2")
            for msl, mp, dst in ((slice(0, 128), 128, q01), (slice(128, 192), 64, q2)):
                ps = ppA.tile([128, 512], F32, tag="bigA", name="psA")[:mp, :QW]
                for o in range(KO):
                    nc.tensor.matmul(ps, wq_s[:, o, msl], xb[:, o, 0:QW],
                                     start=(o == 0), stop=(o == KO - 1))
                nc.vector.tensor_copy(dst[:mp, :], ps)
            st["q01"], st["q2"] = q01, q2

        def _kproj(st, msl, mp, dst):
            # project only the 577 valid tokens; zero the pad columns so the
            # padded score rows multiply to 0 (not NaN) in the mask-mix
            for n0, n1 in ((0, 512), (512, N)):
                ps = ppA.tile([128, 512], F32, tag="bigA", name="psA")[:mp, : n1 - n0]
                for o in range(KO):
                    nc.tensor.matmul(ps, wk_s[:, o, msl], st["xb"][:, o, n0:n1],
                                     start=(o == 0), stop=(o == KO - 1))
                if mp == 128 and n0 != 0:
                    nc.scalar.copy(dst[:mp, n0:n1], ps)
                else:
                    nc.vector.tensor_copy(dst[:mp, n0:n1], ps)
            nc.gpsimd.memset(dst[:mp, N:NP], 0.0)

        def a_k01(st):
            k01 = wpool.tile([128, NP], BF, tag="k01")
            _kproj(st, slice(0, 128), 128, k01)
            st["k01"] = k01

        def a_k2(st):
            k2 = wpool.tile([64, NP], BF, tag="k2")
            _kproj(st, slice(128, 192), 64, k2)
            st["k2"] = k2

        def a_v(st):
            # v-hat (k rows, interleaved [v | ones] columns; ones via memset)
            xb = st["xb"]
            vtb = wpool.tile([128, NJ, NH * VW], BF, tag="vtb")
            for kc in range(NJ):
                ps = ppA.tile([128, 512], F32, tag="bigA", name="psA")[:, : NH * VW]
                for o in range(KO):
                    nc.tensor.matmul(ps, xb[:, o, kc * 128:(kc + 1) * 128], wv_s[:, o, :],
                                     start=(o == 0), stop=(o == KO - 1))
                nc.vector.tensor_copy(vtb[:, kc, :], ps)
            nc.gpsimd.memset(vtb[:, :, HD::VW], 1.0)
            st["vtb"] = vtb

        A_PARTS = (a_q, a_k01, a_k2, a_v)

        def phase_a(b):
            st = {"b": b}
            for p in A_PARTS:
                p(st)
            return st

        def qg(st, g):
            q01, q2 = st["q01"], st["q2"]
            return (q01[0:64], q01[64:128], q2[0:64])[g]

        def kg(st, g):
            k01, k2 = st["k01"], st["k2"]
            return (k01[0:64], k01[64:128], k2[0:64])[g]

        def scores(st):
            # S^T[k, q] per global head, PSUM split 256 + 33 so no matmul
            # output crosses a PSUM bank boundary
            ssb = wpool.tile([128, GH, NJ, QW], BF, tag="ssb")
            for g in range(GH):
                psa = ppSa.tile([128, NJ, 256], F32, tag="sa", name="psSa")
                psb = ppSb.tile([128, NJ, 33], F32, tag="sb", name="psSb")
                for j in range(NJ):
                    kj = kg(st, g)[:, j * 128:(j + 1) * 128]
                    nc.tensor.matmul(psa[:, j, :], kj, qg(st, g)[:, 0:256],
                                     start=True, stop=True)
                    nc.tensor.matmul(psb[:, j, :], kj, qg(st, g)[:, 256:QW],
                                     start=True, stop=True)
                nc.scalar.copy(ssb[:, g, :, 0:256], psa)
                nc.scalar.copy(ssb[:, g, :, 256:QW], psb)
            return ssb

        def mix_exp(ssb, fine=False):
            # mask-mix (attn_h = sum_g ssb_g * mw_gh), split DVE / Pool.
            # fine=True works at j-chunk granularity (same engine split) so
            # the pipeline tail's p@v can start while later chunks mix.
            es = []
            for hh in range(NH):
                at = apool.tile([128, NJ, QW], BF, tag="at")
                t1 = apool.tile([128, NJ, QW], BF, tag="t1")
                t2 = apool.tile([128, NJ, QW], BF, tag="t2")
                e = apool.tile([128, NJ, QW], BF, tag="e")
                jsl = [slice(j, j + 1) for j in range(NJ)] if fine else [slice(0, NJ)]
                for js in jsl:
                    nc.vector.tensor_mul(at[:, js], ssb[:, 0, js], mw_s[:, hh, js])
                    nc.gpsimd.tensor_mul(t1[:, js], ssb[:, 1, js], mw_s[:, NH + hh, js])
                    if hh < 2:
                        nc.gpsimd.tensor_mul(t2[:, js], ssb[:, 2, js],
                                             mw_s[:, 2 * NH + hh, js])
                    else:
                        nc.vector.tensor_mul(t2[:, js], ssb[:, 2, js],
                                             mw_s[:, 2 * NH + hh, js])
                    nc.gpsimd.tensor_tensor(out=at[:, js], in0=at[:, js],
                                            in1=t1[:, js], op=OP.add)
                    nc.vector.tensor_add(at[:, js], at[:, js], t2[:, js])
                    # exp(at - 5): softmax-invariant shift keeps e and Z in
                    # fp16 range on hardware (max logit ~= 11)
                    if not fine:
                        nc.scalar.activation(e[:, 0:4], at[:, 0:4], AF.Exp,
                                             bias=eb_s[:, 1:2])
                        nc.scalar.activation(e[:, 4:5], at[:, 4:5], AF.Exp,
                                             bias=eb_s[:, 0:1])
                    else:
                        bcol = eb_s[:, 0:1] if js.start == 4 else eb_s[:, 1:2]
                        nc.scalar.activation(e[:, js], at[:, js], AF.Exp, bias=bcol)
                es.append(e)
            return es

        def pv(es, vtb):
            # o^T_h (and Z in row 64) = vhat_h^T @ e
            osbz = wpool.tile([VW, NH, QW], BF, tag="osbz")
            for hh in range(NH):
                pov = ppO.tile([VW, QW], F32, tag="ov", name="psO")
                for j in range(NJ):
                    nc.tensor.matmul(pov, vtb[:, j, hh * VW:(hh + 1) * VW],
                                     es[hh][:, j, :],
                                     start=(j == 0), stop=(j == NJ - 1))
                nc.scalar.copy(osbz[:, hh], pov)
            return osbz

        def zout(b, osbz):
            # 1/Z broadcast over the 64 head-dim partitions, per head so the
            # chain pipelines with the per-head pv/osbz copies
            zrec = wpool.tile([1, NH, QW], BF, tag="zrec")
            zrep = wpool.tile([64, NH, QW], BF, tag="zrep")
            on01 = wpool.tile([128, QW], BF, tag="on01")
            on2 = wpool.tile([64, QW], BF, tag="on2")
            ons = (on01[0:64, :], on01[64:128, :], on2[:])
            with nc.allow_low_precision(reason="1/Z in fp16: Z in [1, 640], rel err ~5e-4"):
                nc.vector.reciprocal(zrec[:], osbz[HD:VW, :])
                for hh in range(NH):
                    nc.gpsimd.partition_broadcast(zrep[:, hh], zrec[0:1, hh], channels=64)
                    nc.gpsimd.tensor_mul(ons[hh], osbz[0:HD, hh], zrep[:, hh])

            # output projection (partial, pre-bias; host sums head groups)
            for q0, q1 in ((0, 128), (128, 256), (256, QW)):
                outsb = wpool.tile([128, C], BF, tag="outsb")
                for n0, n1 in ((0, 512), (512, C)):
                    ps = ppA.tile([128, 512], F32, tag="bigA", name="psA")[
                        : q1 - q0, : n1 - n0]
                    nc.tensor.matmul(ps, on01[:, q0:q1], pw01_s[:, n0:n1],
                                     start=True, stop=False)
                    nc.tensor.matmul(ps, on2[:, q0:q1], pw2_s[:, n0:n1],
                                     start=False, stop=True)
                    if n0 == 0:
                        nc.vector.tensor_copy(outsb[: q1 - q0, n0:n1], ps)
                    else:
                        nc.scalar.copy(outsb[: q1 - q0, n0:n1], ps)
                nc.sync.dma_start(out.ap()[b, q0:q1, :], outsb[: q1 - q0, :])

        # depth-4 software pipeline. Iteration b emits
        #   mix+exp(b) | scores(b+2) | phase-A parts | pv(b) | out-proj(b)
        # Scores run two batches ahead of the mix that consumes them, so the
        # mask-mix of b starts right at the top of iteration b, and the PE
        # runs scores/projections while it completes.
        st = {0: phase_a(0), 1: phase_a(1)}
        # big const DMAs ride the sync queue behind xb0/xb1 but ahead of
        # later xb's: g0/g1 mask maps first (needed by the first mix ops)
        nc.sync.dma_start(mw_s[:, 0:2 * NH], mw.ap()[:, 0:2 * NH])
        nc.sync.dma_start(mw_s[:, 2 * NH:], mw.ap()[:, 2 * NH:])
        ssbs = {0: scores(st[0])}
        st[2] = phase_a(2)
        nc.sync.dma_start(pw01_s[:], pw01.ap())
        nc.sync.dma_start(pw2_s[:], pw2.ap())
        ssbs[1] = scores(st[1])
        st[3] = phase_a(3)

        # spread the remaining phase-A parts (batches 4..7) over the
        # iterations, each finished by its scores deadline (iter m-2)
        SPREAD = {
            0: ((4, 0), (4, 1), (4, 2)),
            1: ((4, 3), (5, 0), (5, 1)),
            2: ((5, 2), (5, 3), (6, 0)),
            3: ((6, 1), (6, 2), (6, 3)),
            4: ((7, 0), (7, 1), (7, 2)),
            5: ((7, 3),),
            6: (),
            7: (),
        }
        for b in range(B):
            es = mix_exp(ssbs.pop(b), fine=False)
            for m, pi in SPREAD[b]:
                if pi == 0:
                    st[m] = {"b": m}
                A_PARTS[pi](st[m])
            if b + 2 < B:
                ssbs[b + 2] = scores(st[b + 2])
            osbz = pv(es, st[b]["vtb"])
            del st[b]
            zout(b, osbz)

    nc.compile()
    return nc


def _perms():
    """Per-q-half token permutations: the core's q tokens first."""
    p0 = np.arange(N)
    p1 = np.concatenate([np.arange(QW, N), np.arange(QW)])
    return (p0, p1)


def prep_inputs3(x, masks, Wq, Wk, Wv, mask_proj, proj_w, proj_b):
    f16 = np.float16

    # mask_weights (replicated across batch, per the sharding hint)
    P = (masks.reshape(N * N, ML).astype(np.float32) @ mask_proj).reshape(
        N, N, GH, LH)

    ebp = np.full((128, 2), -5.0, np.float32)
    ebp[VW:, 0] += EXP_NEG  # j=4 chunk: k = 512 + p valid through p = 64

    perms = _perms()
    xtas = []
    mwk = []  # [s] -> [q', k'(pad 640), GH, LH] permuted mask weights
    for s in range(2):
        perm = perms[s]
        xhatT = np.zeros((B, C, NP), np.float32)
        xhatT[:, :, :N] = x[:, perm, :].transpose(0, 2, 1)
        xtas.append(np.ascontiguousarray(
            xhatT.reshape(B, KO, 128, NP).transpose(0, 2, 1, 3)).astype(f16))
        Pp = P[np.ix_(perm, perm)]                      # [q', k', GH, LH]
        Ppad = np.zeros((QW, NP, GH, LH), np.float32)
        Ppad[:, :N] = Pp[:QW]
        mwk.append(Ppad)

    def wpad(w, scale=1.0):
        return np.ascontiguousarray(
            (w * scale).reshape(KO, 128, -1).transpose(1, 0, 2)).astype(f16)

    wqp = wpad(Wq, SCALE)
    wkp = wpad(Wk)

    in_maps = []
    for c in range(8):
        hg, s = c // 2, c % 2
        H0 = NH * hg

        wvh = np.zeros((C, NH * VW), np.float32)
        for hh in range(NH):
            h = H0 + hh
            wvh[:, hh * VW:hh * VW + HD] = Wv[:, h * HD:(h + 1) * HD]
        wvp = np.ascontiguousarray(
            wvh.reshape(KO, 128, -1).transpose(1, 0, 2)).astype(f16)

        pw01p = np.ascontiguousarray(
            proj_w[H0 * HD:(H0 + 2) * HD, :]).astype(f16)
        pw2p = np.ascontiguousarray(
            proj_w[(H0 + 2) * HD:(H0 + 3) * HD, :]).astype(f16)

        # mw[p, g*NH+hh, j, q'] = P[perm[q'], perm[j*128+p], g, H0+hh]
        mwc = np.ascontiguousarray(
            mwk[s][:, :, :, H0:H0 + NH]                 # [q', k', g, hh]
            .reshape(QW, NJ, 128, GH, NH)
            .transpose(2, 3, 4, 1, 0)                   # [p, g, hh, j, q']
            .reshape(128, GH * NH, NJ, QW)).astype(f16)

        in_maps.append({
            "xta": xtas[s],
            "wq": wqp, "wk": wkp, "wv": wvp,
            "pw01": pw01p, "pw2": pw2p,
            "mw": mwc, "eb": ebp,
        })
    return in_maps


_NC3 = None


def get_nc3():
    global _NC3
    if _NC3 is None:
        _NC3 = build_nc3()
    return _NC3


def kernel_v3(x, masks, Wq, Wk, Wv, mask_proj, proj_w, proj_b):
    x = np.asarray(x, np.float32)
    in_maps = prep_inputs3(
        x, np.asarray(masks, np.float32), np.asarray(Wq, np.float32),
        np.asarray(Wk, np.float32), np.asarray(Wv, np.float32),
        np.asarray(mask_proj, np.float32), np.asarray(proj_w, np.float32),
        np.asarray(proj_b, np.float32))
    res = bass_utils.run_bass_kernel_spmd(get_nc3(), in_maps, core_ids=list(range(8)))
    # sum the 4 head-group partials per q-half, concat halves, add bias
    out = np.zeros((B, N, C), np.float32)
    for c in range(8):
        hg, s = c // 2, c % 2
        r = np.asarray(res.results[c]["op"], np.float32)
        if s == 0:
            out[:, 0:QW, :] += r
        else:
            out[:, QW:N, :] += r[:, 0:N - QW, :]
    out += np.asarray(proj_b, np.float32)
    return out.astype(np.float32)


def kernel(x, masks, Wq, Wk, Wv, mask_proj, proj_w, proj_b):
    return kernel_v3(x, masks, Wq, Wk, Wv, mask_proj, proj_w, proj_b)


if __name__ == "__main__":
    rng = np.random.default_rng(0)
    ins = {
        "x": rng.standard_normal((B, N, C)).astype(np.float32),
        "masks": rng.random((N, N, ML)).astype(np.float32),
        "Wq": (rng.standard_normal((C, GH * HD)) * 0.02).astype(np.float32),
        "Wk": (rng.standard_normal((C, GH * HD)) * 0.02).astype(np.float32),
        "Wv": (rng.standard_normal((C, C)) * 0.02).astype(np.float32),
        "mask_proj": (rng.standard_normal((ML, GH * LH)) * 0.5 + 1.0).astype(np.float32),
        "proj_w": (rng.standard_normal((C, C)) * 0.02).astype(np.float32),
        "proj_b": (rng.standard_normal(C) * 0.02).astype(np.float32),
    }
    out = kernel(**ins)
    print(out.shape, out.dtype)
